# revision 4
# baseline (speedup 1.0000x reference)
"""Expert-parallel JetMoE MoE kernel for 8 Trainium2 NeuronCores.

Strategy (per the expert-parallel sharding hint):
- Host computes router logits (fp64 -> fp32), top-2 + softmax gates, and
  dispatches tokens by expert id (the host-side equivalent of the
  wrapper's all-to-all). Router math is ~67 MFLOP - negligible.
- Core e holds expert e's weights (bf16) and runs the SwiGLU MLP over the
  tokens routed to expert e, padded to the max per-expert count C so all
  8 cores run one SPMD NEFF.
- Host combines: out[tok] += gate * y_expert[tok], adds bias.

Device layout: features on partitions, tokens on the free dim, so
mm1 (hT = W1 @ x^T), SwiGLU, and mm2 (yT = W2^T @ aT) chain with no
transposes. Weights are host-retiled so every DMA is fully contiguous.
"""

import json
import os

import ml_dtypes
import numpy as np

_B, _S, _D, _H, _E = 1, 2048, 2048, 5632, 8
_TWO_H = 2 * _H
_P = 128
_KD = _D // _P  # 16 k-tiles over D
_JH = _H // _P  # 44 j-tiles over H (per SwiGLU half)
_ID = _D // _P  # 16 output d-tiles

last_exec_time_ns = None


def _patch_tile_drain():
    """walrus in this image rejects >2 sem waits on one SP Drain; spread the
    TileContext end-of-kernel drain waits across nofuse nops instead."""
    import concourse.mybir as mybir
    import concourse.tile as tile
    from concourse.vector_clock import ScopedClock

    if getattr(tile.TileContext, "_moe_drain_patched", False):
        return

    def _patched(self, tick_clock, wait_clock):
        nop0 = self.nc.sync.nop(nofuse=True, hint="drain_waits")
        wait_clock.add_sem_waits(
            nop0.ins, ScopedClock({None: tick_clock.global_clock})
        )
        si = nop0.ins.sync_info
        waits = list(si.on_wait or [])
        if len(waits) > 1:
            si.on_wait = waits[:1]
            for w in waits[1:]:
                n = self.nc.sync.nop(nofuse=True, hint="drain_waits")
                n.ins.sync_info = mybir.SyncInfo(on_wait=[w], on_update=[])
        self.nc.sync.drain()
        self.nc.all_engine_barrier()
        assert self.sems is not None
        popped = self.nc._tile_sem_poison_stack.pop()
        assert popped is self._sem_poison
        self.nc.clear_and_free_semaphores(list(self.sems.allocated().values()))
        self.nc.all_engine_barrier()

    tile.TileContext._drain_and_barrier = _patched
    tile.TileContext._moe_drain_patched = True


def _fix_bir_waits(bir_bytes):
    """This image's walrus encodes at most 1 sem wait per instruction (2 for
    EventSemaphore); Tile emits more. Splice excess waits onto preceding
    same-engine NoOps."""
    bir = json.loads(bir_bytes)
    for f in bir["functions"]:
        for b in f["blocks"]:
            out = []
            for inst in b["instructions"]:
                si = inst.get("sync_info")
                ow = (si or {}).get("on_wait") or []
                cap = 2 if inst.get("opcode") == "EventSemaphore" else 1
                if len(ow) > cap:
                    for j, w in enumerate(ow[:-cap]):
                        out.append(
                            {
                                "debug": inst.get("debug", {}),
                                "engine": inst["engine"],
                                "ins": [],
                                "outs": [],
                                "name": f'{inst["name"]}-w{j}',
                                "opcode": "NoOp",
                                "text_hint": "waitfix",
                                "sync_info": {"on_wait": [w], "on_update": []},
                            }
                        )
                    si["on_wait"] = ow[-cap:]
                out.append(inst)
            b["instructions"] = out
    return json.dumps(bir).encode()


def _token_chunks(C):
    """Split C tokens into near-equal chunks of <=512 (PSUM fp32 bank limit)."""
    nt = (C + 511) // 512
    base, rem = divmod(C, nt)
    sizes = [base + (1 if i < rem else 0) for i in range(nt)]
    out, t0 = [], 0
    for s in sizes:
        out.append((t0, s))
        t0 += s
    return out


def _build_bass(C):
    import concourse.bass as bass
    import concourse.mybir as mybir
    import concourse.tile as tile

    _patch_tile_drain()
    f32 = mybir.dt.float32
    bf16 = mybir.dt.bfloat16
    chunks = _token_chunks(C)

    nc = bass.Bass()
    xt_d = nc.dram_tensor("xt", [_P, _KD, C], bf16, kind="ExternalInput")
    w1_d = nc.dram_tensor("w1", [2 * _JH, _P, _KD, _P], bf16, kind="ExternalInput")
    w2_d = nc.dram_tensor("w2", [_ID, _P, _JH, _P], bf16, kind="ExternalInput")
    yt_d = nc.dram_tensor("yt", [_ID, _P, C], f32, kind="ExternalOutput")

    with tile.TileContext(nc) as tc:
        with (
            tc.tile_pool(name="xp", bufs=1) as xp,
            tc.tile_pool(name="w1p", bufs=4) as w1p,
            tc.tile_pool(name="w2p", bufs=3) as w2p,
            tc.tile_pool(name="ap", bufs=1) as apool,
            tc.tile_pool(name="sp", bufs=3) as spool,
            tc.tile_pool(name="yp", bufs=3) as ypool,
            tc.tile_pool(name="ps", bufs=4, space=bass.MemorySpace.PSUM) as psp,
            tc.tile_pool(name="ps2", bufs=2, space=bass.MemorySpace.PSUM) as psp2,
        ):
            xt = xp.tile([_P, _KD, C], bf16)
            nc.sync.dma_start(xt[:], xt_d[:])
            at = apool.tile([_P, _JH, C], bf16)

            # ---- mm1 + SwiGLU: at[:, j, t] = silu(hL) * hR ----
            for j in range(_JH):
                wl = w1p.tile([_P, _KD, _P], bf16, tag="w1")
                nc.sync.dma_start(wl[:], w1_d[j])
                wr = w1p.tile([_P, _KD, _P], bf16, tag="w1")
                nc.sync.dma_start(wr[:], w1_d[j + _JH])
                for t0, tn in chunks:
                    psl = psp.tile([_P, tn], f32, tag="ps")
                    for k in range(_KD):
                        nc.tensor.matmul(
                            psl[:],
                            wl[:, k, :],
                            xt[:, k, t0 : t0 + tn],
                            start=(k == 0),
                            stop=(k == _KD - 1),
                        )
                    psr = psp.tile([_P, tn], f32, tag="ps")
                    for k in range(_KD):
                        nc.tensor.matmul(
                            psr[:],
                            wr[:, k, :],
                            xt[:, k, t0 : t0 + tn],
                            start=(k == 0),
                            stop=(k == _KD - 1),
                        )
                    sl = spool.tile([_P, tn], f32, tag="s")
                    nc.scalar.activation(
                        sl[:], psl[:], mybir.ActivationFunctionType.Silu
                    )
                    nc.vector.tensor_mul(at[:, j, t0 : t0 + tn], sl[:], psr[:])

            # ---- mm2: yt[i, :, t] = sum_kh w2[i,:,kh,:]^T @ at[:, kh, t] ----
            for i in range(_ID):
                w2t = w2p.tile([_P, _JH, _P], bf16, tag="w2")
                nc.sync.dma_start(w2t[:], w2_d[i])
                for t0, tn in chunks:
                    ps = psp2.tile([_P, tn], f32, tag="ps2")
                    for kh in range(_JH):
                        nc.tensor.matmul(
                            ps[:],
                            w2t[:, kh, :],
                            at[:, kh, t0 : t0 + tn],
                            start=(kh == 0),
                            stop=(kh == _JH - 1),
                        )
                    yt = ypool.tile([_P, tn], f32, tag="y")
                    nc.scalar.copy(yt[:], ps[:])
                    nc.sync.dma_start(yt_d[i, :, t0 : t0 + tn], yt[:])

    return nc


def kernel(layer_input, router_weight, input_linear_weight, output_linear_weight, bias):
    global last_exec_time_ns
    x = np.ascontiguousarray(np.asarray(layer_input), dtype=np.float32).reshape(
        -1, _D
    )
    rw = np.asarray(router_weight, dtype=np.float32)
    n_tok = x.shape[0]

    # ---- host router: fp64 logits, top-2, softmax gates ----
    logits64 = x.astype(np.float64) @ rw.T.astype(np.float64)
    router_logits = logits64.astype(np.float32)
    order = np.argsort(-logits64, axis=1, kind="stable")
    top2 = order[:, :2]  # [N, 2] expert ids, descending logit
    v = np.take_along_axis(logits64, top2, axis=1)
    g = np.exp(v - v.max(axis=1, keepdims=True))
    gates = (g / g.sum(axis=1, keepdims=True)).astype(np.float32)  # [N, 2]

    # ---- dispatch: token index list per expert ----
    idx_e, gate_e = [], []
    for e in range(_E):
        sel = np.nonzero(top2 == e)
        idx_e.append(sel[0].astype(np.int64))
        gate_e.append(gates[sel[0], sel[1]])
    counts = np.array([len(ix) for ix in idx_e])
    C = int(counts.max())

    bf16 = ml_dtypes.bfloat16
    in_maps = []
    for e in range(_E):
        ix = idx_e[e]
        xe = np.zeros((C, _D), dtype=np.float32)
        xe[: len(ix)] = x[ix]
        # xt[p, k, t] = xe[t, k*128+p]
        xt = np.ascontiguousarray(
            xe.reshape(C, _KD, _P).transpose(2, 1, 0).astype(bf16)
        )
        # w1[j, p, k, m] = W1e[j*128+m, k*128+p]
        w1e = np.asarray(input_linear_weight[e], dtype=np.float32).astype(bf16)
        w1t = np.ascontiguousarray(
            w1e.reshape(2 * _JH, _P, _KD, _P).transpose(0, 3, 2, 1)
        )
        # w2[i, p, kh, m] = Wout[i*128+m, kh*128+p]
        w2e = np.asarray(output_linear_weight[e], dtype=np.float32).astype(bf16)
        w2t = np.ascontiguousarray(
            w2e.reshape(_ID, _P, _JH, _P).transpose(0, 3, 2, 1)
        )
        in_maps.append({"xt": xt, "w1": w1t, "w2": w2t})

    nc = _build_bass(C)
    _orig_tjb = nc.to_json_bytes
    nc.to_json_bytes = lambda: _fix_bir_waits(_orig_tjb())
    from concourse.bass_utils import run_bass_kernel_spmd

    trace = os.environ.get("BASS_MOE_TRACE") == "1"
    kw = {}
    if trace:
        try:
            import ntff_shim  # noqa: F401

            kw = {"trace": True, "trace_cores": list(range(_E))}
        except Exception:
            pass
    res = run_bass_kernel_spmd(nc, in_maps, list(range(_E)), **kw)
    last_exec_time_ns = res.exec_time_ns

    # ---- host combine ----
    out = np.zeros((n_tok, _D), dtype=np.float32)
    for e in range(_E):
        ix = idx_e[e]
        yt = res.results[e]["yt"]  # [16, 128, C] f32, yt[i,p,t] = y[t, i*128+p]
        y = yt.reshape(_D, C).T  # [C, D]
        out[ix] += gate_e[e][:, None] * y[: len(ix)]
    out += np.asarray(bias, dtype=np.float32)[None, :]
    return out.reshape(_B, _S, _D), router_logits


# revision 7
# speedup vs baseline: 1.0066x; 1.0066x over previous
"""Expert-parallel JetMoE MoE kernel for 8 Trainium2 NeuronCores.

Strategy (per the expert-parallel sharding hint):
- Host computes router logits (fp64 -> fp32), top-2 + softmax gates, and
  dispatches tokens by expert id (the host-side equivalent of the
  wrapper's all-to-all). Router math is ~67 MFLOP - negligible.
- Core e holds expert e's weights (bf16) and runs the SwiGLU MLP over the
  tokens routed to expert e, padded to the max per-expert count C so all
  8 cores run one SPMD NEFF.
- Host combines: out[tok] += gate * y_expert[tok], adds bias.

Device layout: features on partitions, tokens on the free dim, so
mm1 (hT = W1 @ x^T), SwiGLU, and mm2 (yT = W2^T @ aT) chain with no
transposes. Weights are host-retiled so every DMA is fully contiguous.
"""

import json
import os

import ml_dtypes
import numpy as np

_B, _S, _D, _H, _E = 1, 2048, 2048, 5632, 8
_TWO_H = 2 * _H
_P = 128
_KD = _D // _P  # 16 k-tiles over D
_JH = _H // _P  # 44 j-tiles over H (per SwiGLU half)
_ID = _D // _P  # 16 output d-tiles

last_exec_time_ns = None


def _patch_tile_drain():
    """walrus in this image rejects >2 sem waits on one SP Drain; spread the
    TileContext end-of-kernel drain waits across nofuse nops instead."""
    import concourse.mybir as mybir
    import concourse.tile as tile
    from concourse.vector_clock import ScopedClock

    if getattr(tile.TileContext, "_moe_drain_patched", False):
        return

    def _patched(self, tick_clock, wait_clock):
        nop0 = self.nc.sync.nop(nofuse=True, hint="drain_waits")
        wait_clock.add_sem_waits(
            nop0.ins, ScopedClock({None: tick_clock.global_clock})
        )
        si = nop0.ins.sync_info
        waits = list(si.on_wait or [])
        if len(waits) > 1:
            si.on_wait = waits[:1]
            for w in waits[1:]:
                n = self.nc.sync.nop(nofuse=True, hint="drain_waits")
                n.ins.sync_info = mybir.SyncInfo(on_wait=[w], on_update=[])
        self.nc.sync.drain()
        self.nc.all_engine_barrier()
        assert self.sems is not None
        popped = self.nc._tile_sem_poison_stack.pop()
        assert popped is self._sem_poison
        self.nc.clear_and_free_semaphores(list(self.sems.allocated().values()))
        self.nc.all_engine_barrier()

    tile.TileContext._drain_and_barrier = _patched
    tile.TileContext._moe_drain_patched = True


def _fix_bir_waits(bir_bytes):
    """This image's walrus encodes at most 1 sem wait per instruction (2 for
    EventSemaphore); Tile emits more. Splice excess waits onto preceding
    same-engine NoOps."""
    bir = json.loads(bir_bytes)
    for f in bir["functions"]:
        for b in f["blocks"]:
            out = []
            for inst in b["instructions"]:
                si = inst.get("sync_info")
                ow = (si or {}).get("on_wait") or []
                cap = 2 if inst.get("opcode") == "EventSemaphore" else 1
                if len(ow) > cap:
                    for j, w in enumerate(ow[:-cap]):
                        out.append(
                            {
                                "debug": inst.get("debug", {}),
                                "engine": inst["engine"],
                                "ins": [],
                                "outs": [],
                                "name": f'{inst["name"]}-w{j}',
                                "opcode": "NoOp",
                                "text_hint": "waitfix",
                                "sync_info": {"on_wait": [w], "on_update": []},
                            }
                        )
                    si["on_wait"] = ow[-cap:]
                out.append(inst)
            b["instructions"] = out
    return json.dumps(bir).encode()


def _token_chunks(C):
    """Split C tokens into near-equal chunks of <=512 (PSUM fp32 bank limit)."""
    nt = (C + 511) // 512
    base, rem = divmod(C, nt)
    sizes = [base + (1 if i < rem else 0) for i in range(nt)]
    out, t0 = [], 0
    for s in sizes:
        out.append((t0, s))
        t0 += s
    return out


def _build_bass(C):
    import concourse.bass as bass
    import concourse.mybir as mybir
    import concourse.tile as tile

    _patch_tile_drain()
    f32 = mybir.dt.float32
    bf16 = mybir.dt.bfloat16
    chunks = _token_chunks(C)

    nc = bass.Bass()
    xt_d = nc.dram_tensor("xt", [_P, _KD, C], bf16, kind="ExternalInput")
    w1_d = nc.dram_tensor("w1", [2 * _JH, _P, _KD, _P], bf16, kind="ExternalInput")
    w2_d = nc.dram_tensor("w2", [_ID, _P, _JH, _P], bf16, kind="ExternalInput")
    yt_d = nc.dram_tensor("yt", [_ID, _P, C], f32, kind="ExternalOutput")

    with tile.TileContext(nc) as tc:
        with (
            tc.tile_pool(name="xp", bufs=1) as xp,
            tc.tile_pool(name="w1p", bufs=4) as w1p,
            tc.tile_pool(name="w2p", bufs=3) as w2p,
            tc.tile_pool(name="ap", bufs=1) as apool,
            tc.tile_pool(name="sp", bufs=3) as spool,
            tc.tile_pool(name="yp", bufs=3) as ypool,
            tc.tile_pool(name="ps", bufs=4, space=bass.MemorySpace.PSUM) as psp,
            tc.tile_pool(name="ps2", bufs=2, space=bass.MemorySpace.PSUM) as psp2,
            tc.tile_pool(name="wps", bufs=1, space=bass.MemorySpace.PSUM) as wpsp,
        ):
            # PE pre-warm: dummy matmuls during the initial input DMA so the
            # HAM clock gate is already at full rate when real work arrives.
            warm = spool.tile([_P, _P], bf16, tag="warm")
            nc.gpsimd.memset(warm[:], 0.0)
            wps = wpsp.tile([_P, 16], f32, tag="warmps")
            for _ in range(30):
                nc.tensor.matmul(
                    wps[:], warm[:], warm[:, :16], start=True, stop=True
                )

            # first weight pair ahead of xt so LDWEIGHTS can issue early
            wl0 = w1p.tile([_P, _KD, _P], bf16, tag="w1")
            nc.sync.dma_start(wl0[:], w1_d[0])
            wr0 = w1p.tile([_P, _KD, _P], bf16, tag="w1")
            nc.sync.dma_start(wr0[:], w1_d[_JH])

            # xt split per k-slice: mm1's k-loop pipelines with the transfer
            xt = xp.tile([_P, _KD, C], bf16)
            for k in range(_KD):
                nc.sync.dma_start(xt[:, k, :], xt_d[:, k, :])
            at = apool.tile([_P, _JH, C], bf16)

            # ---- mm1 + SwiGLU: at[:, j, t] = silu(hL) * hR ----
            for j in range(_JH):
                if j == 0:
                    wl, wr = wl0, wr0
                else:
                    wl = w1p.tile([_P, _KD, _P], bf16, tag="w1")
                    nc.sync.dma_start(wl[:], w1_d[j])
                    wr = w1p.tile([_P, _KD, _P], bf16, tag="w1")
                    nc.sync.dma_start(wr[:], w1_d[j + _JH])
                for t0, tn in chunks:
                    psl = psp.tile([_P, tn], f32, tag="ps")
                    for k in range(_KD):
                        nc.tensor.matmul(
                            psl[:],
                            wl[:, k, :],
                            xt[:, k, t0 : t0 + tn],
                            start=(k == 0),
                            stop=(k == _KD - 1),
                        )
                    psr = psp.tile([_P, tn], f32, tag="ps")
                    for k in range(_KD):
                        nc.tensor.matmul(
                            psr[:],
                            wr[:, k, :],
                            xt[:, k, t0 : t0 + tn],
                            start=(k == 0),
                            stop=(k == _KD - 1),
                        )
                    sl = spool.tile([_P, tn], f32, tag="s")
                    nc.scalar.activation(
                        sl[:], psl[:], mybir.ActivationFunctionType.Silu
                    )
                    nc.vector.tensor_mul(at[:, j, t0 : t0 + tn], sl[:], psr[:])

            # ---- mm2: yt[i, :, t] = sum_kh w2[i,:,kh,:]^T @ at[:, kh, t] ----
            for i in range(_ID):
                w2t = w2p.tile([_P, _JH, _P], bf16, tag="w2")
                nc.sync.dma_start(w2t[:], w2_d[i])
                for t0, tn in chunks:
                    ps = psp2.tile([_P, tn], f32, tag="ps2")
                    for kh in range(_JH):
                        nc.tensor.matmul(
                            ps[:],
                            w2t[:, kh, :],
                            at[:, kh, t0 : t0 + tn],
                            start=(kh == 0),
                            stop=(kh == _JH - 1),
                        )
                    yt = ypool.tile([_P, tn], f32, tag="y")
                    nc.scalar.copy(yt[:], ps[:])
                    nc.sync.dma_start(yt_d[i, :, t0 : t0 + tn], yt[:])

    return nc


def kernel(layer_input, router_weight, input_linear_weight, output_linear_weight, bias):
    global last_exec_time_ns
    x = np.ascontiguousarray(np.asarray(layer_input), dtype=np.float32).reshape(
        -1, _D
    )
    rw = np.asarray(router_weight, dtype=np.float32)
    n_tok = x.shape[0]

    # ---- host router: fp64 logits, top-2, softmax gates ----
    logits64 = x.astype(np.float64) @ rw.T.astype(np.float64)
    router_logits = logits64.astype(np.float32)
    order = np.argsort(-logits64, axis=1, kind="stable")
    top2 = order[:, :2]  # [N, 2] expert ids, descending logit
    v = np.take_along_axis(logits64, top2, axis=1)
    g = np.exp(v - v.max(axis=1, keepdims=True))
    gates = (g / g.sum(axis=1, keepdims=True)).astype(np.float32)  # [N, 2]

    # ---- dispatch: token index list per expert ----
    idx_e, gate_e = [], []
    for e in range(_E):
        sel = np.nonzero(top2 == e)
        idx_e.append(sel[0].astype(np.int64))
        gate_e.append(gates[sel[0], sel[1]])
    counts = np.array([len(ix) for ix in idx_e])
    C = int(counts.max())

    bf16 = ml_dtypes.bfloat16
    in_maps = []
    for e in range(_E):
        ix = idx_e[e]
        xe = np.zeros((C, _D), dtype=np.float32)
        xe[: len(ix)] = x[ix]
        # xt[p, k, t] = xe[t, k*128+p]
        xt = np.ascontiguousarray(
            xe.reshape(C, _KD, _P).transpose(2, 1, 0).astype(bf16)
        )
        # w1[j, p, k, m] = W1e[j*128+m, k*128+p]
        w1e = np.asarray(input_linear_weight[e], dtype=np.float32).astype(bf16)
        w1t = np.ascontiguousarray(
            w1e.reshape(2 * _JH, _P, _KD, _P).transpose(0, 3, 2, 1)
        )
        # w2[i, p, kh, m] = Wout[i*128+m, kh*128+p]
        w2e = np.asarray(output_linear_weight[e], dtype=np.float32).astype(bf16)
        w2t = np.ascontiguousarray(
            w2e.reshape(_ID, _P, _JH, _P).transpose(0, 3, 2, 1)
        )
        in_maps.append({"xt": xt, "w1": w1t, "w2": w2t})

    nc = _build_bass(C)
    _orig_tjb = nc.to_json_bytes
    nc.to_json_bytes = lambda: _fix_bir_waits(_orig_tjb())
    from concourse.bass_utils import run_bass_kernel_spmd

    trace = os.environ.get("BASS_MOE_TRACE") == "1"
    kw = {}
    if trace:
        try:
            import ntff_shim  # noqa: F401

            kw = {"trace": True, "trace_cores": list(range(_E))}
        except Exception:
            pass
    res = run_bass_kernel_spmd(nc, in_maps, list(range(_E)), **kw)
    last_exec_time_ns = res.exec_time_ns

    # ---- host combine ----
    out = np.zeros((n_tok, _D), dtype=np.float32)
    for e in range(_E):
        ix = idx_e[e]
        yt = res.results[e]["yt"]  # [16, 128, C] f32, yt[i,p,t] = y[t, i*128+p]
        y = yt.reshape(_D, C).T  # [C, D]
        out[ix] += gate_e[e][:, None] * y[: len(ix)]
    out += np.asarray(bias, dtype=np.float32)[None, :]
    return out.reshape(_B, _S, _D), router_logits
